# revision 9
# baseline (speedup 1.0000x reference)
"""Entropy-regularized attention (standard MHA fwd) on 8 trn2 cores.

Sharding: core c -> batch b=c//4, head-group g=c%4 (4 of 16 heads).
Each core computes q/k/v for its 256-wide head-group slice, transposed-
layout attention (scores^T = K^T-stationary matmuls, exp on ACT, AV with
v-stationary producing avT), then a row-split Wo partial product.
Host sums the 4 partials per batch and adds bo (the "all-reduce").

v2 restructure vs baseline:
- x / Wq / Wk / Wv shipped as bf16 (halves input DMA), xT DMA'd in
  4 column blocks so the first projection starts ~15us earlier.
- Attention loop is software-pipelined: scores+exp for tile i+1 are
  emitted before AV of tile i, so the PE never sits on the exp latency
  and the exp pipeline (ACT) stays fed.
- All fill work (v projections, mt1 q/k projections, Wo partials of the
  previous query-group, softmax-normalize) lives in a deferred queue
  popped one small closure per loop iteration.
- Softmax denominators of all 4 heads are normalized with ONE Ln + ONE
  Exp on a [4, 1024] tile (ACT cost is free-size-driven, so per-head
  [1,1024] activations waste 4x ACT time).
- SBUF->SBUF staging copies run on the Pool engine (nc.gpsimd); PSUM
  evictions must stay on DVE (GPSIMD cannot access PSUM).
"""

import sys

for _p in ("/opt/trn_rl_repo", "/root/.axon_site/_ro/trn_rl_repo"):
    if _p not in sys.path:
        sys.path.insert(0, _p)

import numpy as np

import concourse.bass as bass
import concourse.mybir as mybir
import concourse.tile as tile
from concourse import bacc

P = 128
S = 2048  # sequence length
D = 1024  # hidden
DG = 256  # per-core head-group width (4 heads x 64)
HD = 64
NHL = 4  # heads per core
KT_D = D // P  # 8 contraction tiles for projections
ST = S // P  # 16 sequence tiles
QG = 1024  # qi group size (PSUM budget: scores 2x2 banks + av 2 + ops 2)
NQG = S // QG

F32 = mybir.dt.float32
F32R = mybir.dt.float32r
BF16 = mybir.dt.bfloat16


def build_nc():
    nc = bacc.Bacc(None, target_bir_lowering=False)

    xT = nc.dram_tensor("xT", [D, S], BF16, kind="ExternalInput")
    wq = nc.dram_tensor("wq", [D, DG], BF16, kind="ExternalInput")
    wk = nc.dram_tensor("wk", [D, DG], BF16, kind="ExternalInput")
    wv = nc.dram_tensor("wv", [D, DG], BF16, kind="ExternalInput")
    wo = nc.dram_tensor("wo", [DG, D], F32R, kind="ExternalInput")
    bq = nc.dram_tensor("bq", [P, 2], F32, kind="ExternalInput")
    bk = nc.dram_tensor("bk", [P, 2], F32, kind="ExternalInput")
    bv = nc.dram_tensor("bv", [1, DG], F32R, kind="ExternalInput")
    out = nc.dram_tensor("out", [S, D], F32, kind="ExternalOutput")

    with tile.TileContext(nc) as tc:
        _body(tc, nc, xT, wq, wk, wv, wo, bq, bk, bv, out)

    # Pin Exp/Ln to the one table set holding both: strip them from the
    # competing sets (dict order and size preserved, so act_func_set_id
    # indices stay valid). Without this the table-load pass alternates
    # exp_and_others <-> natural_log per head (~17 x 1.3us + PE stalls).
    import concourse.bacc as _bacc_mod

    _orig_tables = _bacc_mod.get_activation_tables

    def _pinned_tables(arch):
        t = _orig_tables(arch)
        for name, fns in t.items():
            if name != "natural_log_exp_and_others":
                fns.discard(mybir.ActivationFunctionType.Exp)
                fns.discard(mybir.ActivationFunctionType.Ln)
        return t

    _bacc_mod.get_activation_tables = _pinned_tables
    try:
        nc.compile()
    finally:
        _bacc_mod.get_activation_tables = _orig_tables
    return nc


def _body(tc, nc, xT, wq, wk, wv, wo, bq, bk, bv, out):
    from collections import deque
    from contextlib import ExitStack

    with ExitStack() as ctx:
        ctx.enter_context(
            nc.allow_low_precision(
                reason="bf16 matmul inputs; accumulation is fp32 PSUM"
            )
        )
        persist = ctx.enter_context(tc.tile_pool(name="persist", bufs=1))
        expool = ctx.enter_context(tc.tile_pool(name="expool", bufs=4))
        npool = ctx.enter_context(tc.tile_pool(name="npool", bufs=2))
        opool = ctx.enter_context(tc.tile_pool(name="opool", bufs=3))
        # PSUM budget (8 banks): sc 2x[128,1024]=4, av 1x[128,1024]=2,
        # ops 2x[128,512]=2. qkv/v/rb/proj tiles all use the ops slots.
        ps_sc = ctx.enter_context(tc.tile_pool(name="ps_sc", bufs=2, space="PSUM"))
        ps_av = ctx.enter_context(tc.tile_pool(name="ps_av", bufs=1, space="PSUM"))
        ps_o = ctx.enter_context(tc.tile_pool(name="ps_o", bufs=2, space="PSUM"))

        qT_sb = persist.tile([P, 2, S], BF16)
        kT_sb = persist.tile([P, 2, S], BF16)
        v_sb = persist.tile([P, ST, NHL * 65], BF16)  # 65-striped: col 64 = ones
        avT = [
            persist.tile([P, 2, QG], F32R, tag=f"avT{g}", name=f"avT{g}")
            for g in range(NQG)
        ]
        wo_sb = persist.tile([P, 2, D], F32R)
        ones_row = persist.tile([1, P], F32R)
        xT_sb = persist.tile([P, KT_D, S], BF16)
        wq_sb = persist.tile([P, KT_D, DG], BF16, tag="wq")
        wk_sb = persist.tile([P, KT_D, DG], BF16, tag="wk")
        wv_sb = persist.tile([P, KT_D, DG], BF16, tag="wv")
        bq_sb = persist.tile([P, 2], F32, tag="bq")
        bk_sb = persist.tile([P, 2], F32, tag="bk")
        bv_sb = persist.tile([1, DG], F32R, tag="bv")

        # DMAs ordered to match first-use: wk then the first xT column
        # block gate the first projection chain.
        nc.sync.dma_start(bq_sb[:], bq[:])
        nc.sync.dma_start(bk_sb[:], bk[:])
        nc.sync.dma_start(bv_sb[:], bv[:])
        xTr = xT.rearrange("(kt p) s -> p kt s", p=P)
        nc.sync.dma_start(wk_sb[:], wk.rearrange("(kt p) n -> p kt n", p=P))
        nc.sync.dma_start(xT_sb[:, :, 0:512], xTr[:, :, 0:512])
        nc.sync.dma_start(wq_sb[:], wq.rearrange("(kt p) n -> p kt n", p=P))
        nc.sync.dma_start(xT_sb[:, :, 512:1024], xTr[:, :, 512:1024])
        nc.sync.dma_start(wv_sb[:], wv.rearrange("(kt p) n -> p kt n", p=P))
        nc.sync.dma_start(xT_sb[:, :, 1024:1536], xTr[:, :, 1024:1536])
        nc.sync.dma_start(xT_sb[:, :, 1536:2048], xTr[:, :, 1536:2048])
        nc.sync.dma_start(wo_sb[:], wo.rearrange("(kt p) n -> p kt n", p=P))

        # memset can't emit float32r; stage fp32 ones and copy-cast (rounds)
        ones_f32 = persist.tile([P, P], F32)
        nc.vector.memset(ones_f32[:], 1.0)
        nc.vector.tensor_copy(ones_row[:], ones_f32[0:1, :])
        ones_all = persist.tile([P, P], F32R)
        nc.vector.tensor_copy(ones_all[:], ones_f32[:])
        nc.vector.tensor_copy(
            v_sb.rearrange("p st (h w) -> p st h w", w=65)[:, :, :, 64],
            ones_f32[:, 0:64].rearrange("p (st h) -> p st h", h=4),
        )

        # ---- projection building blocks ----
        def qk_half(ps, wsb, mt, nq, kts):
            for kt in kts:
                nc.tensor.matmul(
                    ps[:, 0:512],
                    wsb[:, kt, mt * P : (mt + 1) * P],
                    xT_sb[:, kt, nq * 512 : (nq + 1) * 512],
                    start=(kt == 0),
                    stop=(kt == KT_D - 1),
                )

        def qk_evict(ps, bsb, dest, mt, nq):
            nc.vector.tensor_scalar_add(
                dest[:, mt, nq * 512 : (nq + 1) * 512],
                ps[:, 0:512],
                bsb[:, mt : mt + 1],
            )

        def qk_full(wsb, bsb, dest, mt, nq):
            ps = ps_o.tile([P, 512], F32, tag="ops", name="qkps")
            qk_half(ps, wsb, mt, nq, range(0, KT_D))
            qk_evict(ps, bsb, dest, mt, nq)

        def qk_deferred(wsb, bsb, dest, mt, nq):
            # two closures: kt 0..3, then kt 4..7 + bias eviction
            box = []

            def first():
                ps = ps_o.tile([P, 512], F32, tag="ops", name="qkps")
                qk_half(ps, wsb, mt, nq, range(0, KT_D // 2))
                box.append(ps)

            def second():
                ps = box.pop()
                qk_half(ps, wsb, mt, nq, range(KT_D // 2, KT_D))
                qk_evict(ps, bsb, dest, mt, nq)

            return [first, second]

        def v_group(st):
            ps = ps_o.tile([P, 512], F32, tag="ops", name="vps")
            for kt in range(KT_D):
                nc.tensor.matmul(
                    ps[:, 0:DG],
                    xT_sb[:, kt, st * P : (st + 1) * P],
                    wv_sb[:, kt, :],
                    start=(kt == 0),
                    stop=False,
                )
            nc.tensor.matmul(
                ps[:, 0:DG],
                ones_row[0:1, 0:P],
                bv_sb[0:1, :],
                start=False,
                stop=True,
            )
            nc.vector.tensor_copy(
                v_sb.rearrange("p st (h w) -> p st h w", w=65)[:, st, :, 0:64],
                ps[:, 0:DG].rearrange("p (h w) -> p h w", w=64),
            )

        # Upfront (overlaps input DMA): everything head 0/1 of qg0 needs
        # except v st>=2, which streams in through the deferred queue.
        qk_full(wk_sb, bk_sb, kT_sb, 0, 0)
        qk_full(wq_sb, bq_sb, qT_sb, 0, 0)
        v_group(0)
        v_group(1)
        qk_full(wk_sb, bk_sb, kT_sb, 0, 1)
        qk_full(wq_sb, bq_sb, qT_sb, 0, 1)
        qk_full(wk_sb, bk_sb, kT_sb, 0, 2)
        qk_full(wk_sb, bk_sb, kT_sb, 0, 3)

        deferred = deque()
        for st in range(2, ST):  # units 0..13 (v st ready 2 units early)
            deferred.append(lambda st=st: v_group(st))
        for nq in range(4):  # units 14..21
            deferred.extend(qk_deferred(wk_sb, bk_sb, kT_sb, 1, nq))
        for nq in range(2):  # units 22..25
            deferred.extend(qk_deferred(wq_sb, bq_sb, qT_sb, 1, nq))
        for nq in range(2, 4):  # units 26..29
            deferred.extend(qk_deferred(wq_sb, bq_sb, qT_sb, 0, nq))
        for nq in range(2, 4):  # units 30..33
            deferred.extend(qk_deferred(wq_sb, bq_sb, qT_sb, 1, nq))

        # ---- attention phase (software-pipelined) ----
        def emit_sc_exp(qg, h, kt):
            mt, po = h // 2, (h % 2) * 64
            q0 = qg * QG
            sc = ps_sc.tile([P, QG], F32, tag="sc", name="sc")
            for nq in range(QG // 512):
                nc.tensor.matmul(
                    sc[:, nq * 512 : (nq + 1) * 512],
                    kT_sb[po : po + 64, mt, kt * P : (kt + 1) * P],
                    qT_sb[
                        po : po + 64,
                        mt,
                        q0 + nq * 512 : q0 + (nq + 1) * 512,
                    ],
                    start=True,
                    stop=True,
                )
            ex = expool.tile([P, QG], BF16, tag="ex", name="ex")
            nc.scalar.activation(
                ex[:], sc[:], mybir.ActivationFunctionType.Exp, scale=0.125
            )
            return ex

        uls = {}
        l4s = {}

        def finish_head(qg, h, av):
            # single copy releases the av PSUM banks; row 64 is the
            # softmax denominator l (from the ones column of v_sb).
            # Head h's l goes to partition 32h so the rb broadcast matmul
            # can use a matching base partition (must be 0/32/64/96).
            if h == 0:
                l4s[qg] = npool.tile([97, QG], F32, tag="l4", name="l4")
                nc.gpsimd.memset(l4s[qg][:], 1.0)
            ul = npool.tile([65, QG], F32R, tag="ul", bufs=4, name="ul")
            nc.vector.tensor_copy(ul[:], av[0:65, :])
            nc.gpsimd.tensor_copy(
                l4s[qg][32 * h : 32 * h + 1, :], ul[64:65, :]
            )
            uls[(qg, h)] = ul

        def rb_mult(qg, h, r4):
            mt, po = h // 2, (h % 2) * 64
            ul = uls.pop((qg, h))
            if h == 3:
                # matmul base partitions may only be 0/32/64; stage h3's
                # reciprocal row down to partition 0
                r3 = npool.tile([1, QG], F32R, tag="r3", name="r3")
                nc.gpsimd.tensor_copy(r3[:], r4[96:97, :])
                r_ap, base = r3, 0
            else:
                r_ap, base = r4, 32 * h
            for nq in range(QG // 512):
                rb = ps_o.tile([P, 512], F32, tag="ops", name="rb")
                nc.tensor.matmul(
                    rb[0:64, :],
                    ones_all[base : base + 1, 0:64],
                    r_ap[base : base + 1, nq * 512 : (nq + 1) * 512],
                    start=True,
                    stop=True,
                )
                nc.vector.tensor_mul(
                    out=avT[qg][po : po + 64, mt, nq * 512 : (nq + 1) * 512],
                    in0=ul[0:64, nq * 512 : (nq + 1) * 512],
                    in1=rb[0:64, :],
                )

        def phase_d(qg, sti):
            st = qg * (QG // P) + sti
            ot = opool.tile([P, D], F32, tag="ot", name="ot")
            for nd in range(2):
                pp = ps_o.tile([P, 512], F32, tag="ops", name="pp")
                for kt2 in range(2):
                    nc.tensor.matmul(
                        pp[:],
                        avT[qg][:, kt2, sti * P : (sti + 1) * P],
                        wo_sb[:, kt2, nd * 512 : (nd + 1) * 512],
                        start=(kt2 == 0),
                        stop=(kt2 == 1),
                    )
                nc.vector.tensor_copy(ot[:, nd * 512 : (nd + 1) * 512], pp[:])
            nc.sync.dma_start(out[st * P : (st + 1) * P, :], ot[:])

        def make_norm(qg):
            def fire():
                # one Ln + one Exp for all 4 heads' denominators:
                # 1/l = exp(-ln(l)); ACT cost is free-size-driven so the
                # [97,1024] batch costs the same as a single [1,1024].
                ls = npool.tile([97, QG], F32, tag="ls", name="ls")
                nc.scalar.activation(
                    ls[:], l4s[qg][:], mybir.ActivationFunctionType.Ln
                )
                r4 = npool.tile([97, QG], F32R, tag="r4", name="r4")
                nc.scalar.activation(
                    r4[:], ls[:], mybir.ActivationFunctionType.Exp, scale=-1.0
                )
                for h in reversed(range(NHL)):
                    deferred.appendleft(lambda h=h: rb_mult(qg, h, r4))
                for sti in range(QG // P):
                    deferred.append(lambda sti=sti: phase_d(qg, sti))

            return fire

        units = [
            (qg, h, kt)
            for qg in range(NQG)
            for h in range(NHL)
            for kt in range(ST)
        ]
        ex_tiles = {0: emit_sc_exp(*units[0])}
        av = None
        pending_norm = None
        for i, (qg, h, kt) in enumerate(units):
            if i + 1 < len(units):
                ex_tiles[i + 1] = emit_sc_exp(*units[i + 1])
            if pending_norm is not None and kt == 3:
                pending_norm()
                pending_norm = None
            if deferred:
                deferred.popleft()()
            if kt == 0:
                av = ps_av.tile([P, QG], F32, tag="av", name="av")
            ex = ex_tiles.pop(i)
            for nq in range(QG // 512):
                nc.tensor.matmul(
                    av[0:65, nq * 512 : (nq + 1) * 512],
                    v_sb[:, kt, h * 65 : h * 65 + 65],
                    ex[:, nq * 512 : (nq + 1) * 512],
                    start=(kt == 0),
                    stop=(kt == ST - 1),
                )
            if kt == ST - 1:
                finish_head(qg, h, av)
                if h == NHL - 1:
                    pending_norm = make_norm(qg)

        # tail: last qg's norm + Wo partials
        pending_norm()
        while deferred:
            deferred.popleft()()


_NC_CACHE = None


def get_nc():
    global _NC_CACHE
    if _NC_CACHE is None:
        _NC_CACHE = build_nc()
    return _NC_CACHE


def make_in_maps(x, Wq, bq, Wk, bk, Wv, bv, Wo, bo):
    import ml_dtypes

    bf16 = ml_dtypes.bfloat16
    in_maps = []
    for c in range(8):
        b, g = c // 4, c % 4
        sl = slice(g * DG, (g + 1) * DG)
        in_maps.append(
            {
                "xT": np.ascontiguousarray(x[b].T.astype(bf16)),
                "wq": np.ascontiguousarray(Wq[:, sl].astype(bf16)),
                "wk": np.ascontiguousarray(Wk[:, sl].astype(bf16)),
                "wv": np.ascontiguousarray(Wv[:, sl].astype(bf16)),
                "wo": np.ascontiguousarray(Wo[sl, :]),
                "bq": np.ascontiguousarray(bq[sl].reshape(2, P).T),
                "bk": np.ascontiguousarray(bk[sl].reshape(2, P).T),
                "bv": np.ascontiguousarray(bv[sl].reshape(1, DG)),
            }
        )
    return in_maps


def kernel(x, Wq, bq, Wk, bk, Wv, bv, Wo, bo, _run_kwargs=None):
    from concourse.bass_utils import run_bass_kernel_spmd

    x = np.asarray(x, dtype=np.float32)
    nc = get_nc()
    in_maps = make_in_maps(
        x,
        np.asarray(Wq, np.float32),
        np.asarray(bq, np.float32),
        np.asarray(Wk, np.float32),
        np.asarray(bk, np.float32),
        np.asarray(Wv, np.float32),
        np.asarray(bv, np.float32),
        np.asarray(Wo, np.float32),
        np.asarray(bo, np.float32),
    )
    res = run_bass_kernel_spmd(
        nc, in_maps, core_ids=list(range(8)), **(_run_kwargs or {})
    )
    bo = np.asarray(bo, np.float32)
    outp = np.empty((2, S, D), dtype=np.float32)
    for b in range(2):
        acc = res.results[4 * b]["out"].astype(np.float32)
        for g in range(1, 4):
            acc = acc + res.results[4 * b + g]["out"]
        outp[b] = acc + bo[None, :]
    kernel.last_result = res
    return outp


# revision 13
# speedup vs baseline: 1.0119x; 1.0119x over previous
"""Entropy-regularized attention (standard MHA fwd) on 8 trn2 cores.

Sharding: core c -> batch b=c//4, head-group g=c%4 (4 of 16 heads).
Each core computes q/k/v for its 256-wide head-group slice, transposed-
layout attention (scores^T = K^T-stationary matmuls, exp on ACT, AV with
v-stationary producing avT), then a row-split Wo partial product.
Host sums the 4 partials per batch and adds bo (the "all-reduce").

v2 restructure vs baseline:
- x / Wq / Wk / Wv shipped as bf16 (halves input DMA), xT DMA'd in
  4 column blocks so the first projection starts ~15us earlier.
- Attention loop is software-pipelined: scores+exp for tile i+1 are
  emitted before AV of tile i, so the PE never sits on the exp latency
  and the exp pipeline (ACT) stays fed.
- All fill work (v projections, mt1 q/k projections, Wo partials of the
  previous query-group, softmax-normalize) lives in a deferred queue
  popped one small closure per loop iteration.
- Softmax denominators of all 4 heads are normalized with ONE Ln + ONE
  Exp on a [4, 1024] tile (ACT cost is free-size-driven, so per-head
  [1,1024] activations waste 4x ACT time).
- SBUF->SBUF staging copies run on the Pool engine (nc.gpsimd); PSUM
  evictions must stay on DVE (GPSIMD cannot access PSUM).
"""

import sys

for _p in ("/opt/trn_rl_repo", "/root/.axon_site/_ro/trn_rl_repo"):
    if _p not in sys.path:
        sys.path.insert(0, _p)

import numpy as np

import concourse.bass as bass
import concourse.mybir as mybir
import concourse.tile as tile
from concourse import bacc

P = 128
S = 2048  # sequence length
D = 1024  # hidden
DG = 256  # per-core head-group width (4 heads x 64)
HD = 64
NHL = 4  # heads per core
KT_D = D // P  # 8 contraction tiles for projections
ST = S // P  # 16 sequence tiles
QG = 1024  # qi group size (PSUM budget: scores 2x2 banks + av 2 + ops 2)
NQG = S // QG

F32 = mybir.dt.float32
F32R = mybir.dt.float32r
BF16 = mybir.dt.bfloat16


def build_nc():
    nc = bacc.Bacc(None, target_bir_lowering=False)

    # all inputs pre-arranged on the host into the exact SBUF layouts so
    # every DMA is a contiguous-per-partition blob (large descriptors)
    xT = nc.dram_tensor("xT", [P, 4 * KT_D * 512], BF16, kind="ExternalInput")
    wq = nc.dram_tensor("wq", [P, KT_D * DG], BF16, kind="ExternalInput")
    wk = nc.dram_tensor("wk", [P, KT_D * DG], BF16, kind="ExternalInput")
    wv = nc.dram_tensor("wv", [P, KT_D * DG], BF16, kind="ExternalInput")
    wo = nc.dram_tensor("wo", [P, 2 * D], F32R, kind="ExternalInput")
    bq = nc.dram_tensor("bq", [P, 2], F32, kind="ExternalInput")
    bk = nc.dram_tensor("bk", [P, 2], F32, kind="ExternalInput")
    bv = nc.dram_tensor("bv", [1, DG], F32R, kind="ExternalInput")
    out = nc.dram_tensor("out", [S, D], BF16, kind="ExternalOutput")

    with tile.TileContext(nc) as tc:
        _body(tc, nc, xT, wq, wk, wv, wo, bq, bk, bv, out)

    # Pin Exp/Ln to the one table set holding both: strip them from the
    # competing sets (dict order and size preserved, so act_func_set_id
    # indices stay valid). Without this the table-load pass alternates
    # exp_and_others <-> natural_log per head (~17 x 1.3us + PE stalls).
    import concourse.bacc as _bacc_mod

    _orig_tables = _bacc_mod.get_activation_tables

    def _pinned_tables(arch):
        t = _orig_tables(arch)
        for name, fns in t.items():
            if name != "natural_log_exp_and_others":
                fns.discard(mybir.ActivationFunctionType.Exp)
                fns.discard(mybir.ActivationFunctionType.Ln)
        return t

    _bacc_mod.get_activation_tables = _pinned_tables
    try:
        nc.compile()
    finally:
        _bacc_mod.get_activation_tables = _orig_tables
    return nc


def _body(tc, nc, xT, wq, wk, wv, wo, bq, bk, bv, out):
    from collections import deque
    from contextlib import ExitStack

    with ExitStack() as ctx:
        ctx.enter_context(
            nc.allow_low_precision(
                reason="bf16 matmul inputs; accumulation is fp32 PSUM"
            )
        )
        persist = ctx.enter_context(tc.tile_pool(name="persist", bufs=1))
        expool = ctx.enter_context(tc.tile_pool(name="expool", bufs=4))
        npool = ctx.enter_context(tc.tile_pool(name="npool", bufs=2))
        opool = ctx.enter_context(tc.tile_pool(name="opool", bufs=3))
        # PSUM budget (8 banks): sc 2x[128,1024]=4, av 1x[128,1024]=2,
        # ops 2x[128,512]=2. qkv/v/rb/proj tiles all use the ops slots.
        ps_sc = ctx.enter_context(tc.tile_pool(name="ps_sc", bufs=2, space="PSUM"))
        ps_av = ctx.enter_context(tc.tile_pool(name="ps_av", bufs=1, space="PSUM"))
        ps_o = ctx.enter_context(tc.tile_pool(name="ps_o", bufs=2, space="PSUM"))

        qT_sb = persist.tile([P, 2, S], BF16)
        kT_sb = persist.tile([P, 2, S], BF16)
        v_sb = persist.tile([P, ST, NHL * 65], BF16)  # 65-striped: col 64 = ones
        avT = [
            persist.tile([P, 2, QG], F32R, tag=f"avT{g}", name=f"avT{g}")
            for g in range(NQG)
        ]
        wo_sb = persist.tile([P, 2, D], F32R)
        ones_row = persist.tile([1, P], F32R)
        xT_sb = persist.tile([P, 4, KT_D, 512], BF16)
        wq_sb = persist.tile([P, KT_D, DG], BF16, tag="wq")
        wk_sb = persist.tile([P, KT_D, DG], BF16, tag="wk")
        wv_sb = persist.tile([P, KT_D, DG], BF16, tag="wv")
        bq_sb = persist.tile([P, 2], F32, tag="bq")
        bk_sb = persist.tile([P, 2], F32, tag="bk")
        bv_sb = persist.tile([1, DG], F32R, tag="bv")

        # DMAs ordered to match first-use: wk then the first xT column
        # block gate the first projection chain. All sources are
        # host-prearranged so each transfer is contiguous per partition.
        xTr = xT.rearrange("p (cb kt s) -> p cb kt s", kt=KT_D, s=512)
        nc.sync.dma_start(wk_sb[:], wk.rearrange("p (kt n) -> p kt n", n=DG))
        nc.sync.dma_start(xT_sb[:, 0], xTr[:, 0])
        nc.sync.dma_start(wq_sb[:], wq.rearrange("p (kt n) -> p kt n", n=DG))
        nc.sync.dma_start(wv_sb[:], wv.rearrange("p (kt n) -> p kt n", n=DG))
        nc.sync.dma_start(bq_sb[:], bq[:])
        nc.sync.dma_start(bk_sb[:], bk[:])
        nc.sync.dma_start(bv_sb[:], bv[:])
        nc.sync.dma_start(xT_sb[:, 1], xTr[:, 1])
        nc.sync.dma_start(xT_sb[:, 2], xTr[:, 2])
        nc.sync.dma_start(xT_sb[:, 3], xTr[:, 3])
        nc.sync.dma_start(wo_sb[:], wo.rearrange("p (kt n) -> p kt n", n=D))

        # memset can't emit float32r; stage fp32 ones and copy-cast (rounds)
        ones_f32 = persist.tile([P, P], F32)
        nc.vector.memset(ones_f32[:], 1.0)
        nc.vector.tensor_copy(ones_row[:], ones_f32[0:1, :])
        ones_all = persist.tile([P, P], F32R)
        nc.vector.tensor_copy(ones_all[:], ones_f32[:])
        nc.vector.tensor_copy(
            v_sb.rearrange("p st (h w) -> p st h w", w=65)[:, :, :, 64],
            ones_f32[:, 0:64].rearrange("p (st h) -> p st h", h=4),
        )

        # ---- projection building blocks ----
        def qk_half(ps, wsb, mt, nq, kts):
            for kt in kts:
                nc.tensor.matmul(
                    ps[:, 0:512],
                    wsb[:, kt, mt * P : (mt + 1) * P],
                    xT_sb[:, nq, kt, :],
                    start=(kt == 0),
                    stop=(kt == KT_D - 1),
                )

        def qk_evict(ps, bsb, dest, mt, nq):
            nc.vector.tensor_scalar_add(
                dest[:, mt, nq * 512 : (nq + 1) * 512],
                ps[:, 0:512],
                bsb[:, mt : mt + 1],
            )

        def qk_full(wsb, bsb, dest, mt, nq):
            ps = ps_o.tile([P, 512], F32, tag="ops", name="qkps")
            qk_half(ps, wsb, mt, nq, range(0, KT_D))
            qk_evict(ps, bsb, dest, mt, nq)

        def qk_deferred(wsb, bsb, dest, mt, nq):
            # two closures: kt 0..3, then kt 4..7 + bias eviction
            box = []

            def first():
                ps = ps_o.tile([P, 512], F32, tag="ops", name="qkps")
                qk_half(ps, wsb, mt, nq, range(0, KT_D // 2))
                box.append(ps)

            def second():
                ps = box.pop()
                qk_half(ps, wsb, mt, nq, range(KT_D // 2, KT_D))
                qk_evict(ps, bsb, dest, mt, nq)

            return [first, second]

        def v_group(st):
            ps = ps_o.tile([P, 512], F32, tag="ops", name="vps")
            for kt in range(KT_D):
                nc.tensor.matmul(
                    ps[:, 0:DG],
                    xT_sb[:, st // 4, kt, (st % 4) * P : (st % 4 + 1) * P],
                    wv_sb[:, kt, :],
                    start=(kt == 0),
                    stop=False,
                )
            nc.tensor.matmul(
                ps[:, 0:DG],
                ones_row[0:1, 0:P],
                bv_sb[0:1, :],
                start=False,
                stop=True,
            )
            nc.vector.tensor_copy(
                v_sb.rearrange("p st (h w) -> p st h w", w=65)[:, st, :, 0:64],
                ps[:, 0:DG].rearrange("p (h w) -> p h w", w=64),
            )

        # Upfront (overlaps input DMA): everything head 0/1 of qg0 needs
        # except v st>=2, which streams in through the deferred queue.
        qk_full(wk_sb, bk_sb, kT_sb, 0, 0)
        qk_full(wq_sb, bq_sb, qT_sb, 0, 0)
        v_group(0)
        v_group(1)
        qk_full(wk_sb, bk_sb, kT_sb, 0, 1)
        qk_full(wq_sb, bq_sb, qT_sb, 0, 1)
        qk_full(wk_sb, bk_sb, kT_sb, 0, 2)
        qk_full(wk_sb, bk_sb, kT_sb, 0, 3)

        deferred = deque()
        for st in range(2, ST):  # units 0..13 (v st ready 2 units early)
            deferred.append(lambda st=st: v_group(st))
        for nq in range(4):  # units 14..21
            deferred.extend(qk_deferred(wk_sb, bk_sb, kT_sb, 1, nq))
        for nq in range(2):  # units 22..25
            deferred.extend(qk_deferred(wq_sb, bq_sb, qT_sb, 1, nq))
        for nq in range(2, 4):  # units 26..29
            deferred.extend(qk_deferred(wq_sb, bq_sb, qT_sb, 0, nq))
        for nq in range(2, 4):  # units 30..33
            deferred.extend(qk_deferred(wq_sb, bq_sb, qT_sb, 1, nq))

        # ---- attention phase (software-pipelined) ----
        def emit_sc_exp(qg, h, kt):
            mt, po = h // 2, (h % 2) * 64
            q0 = qg * QG
            sc = ps_sc.tile([P, QG], F32, tag="sc", name="sc")
            for nq in range(QG // 512):
                nc.tensor.matmul(
                    sc[:, nq * 512 : (nq + 1) * 512],
                    kT_sb[po : po + 64, mt, kt * P : (kt + 1) * P],
                    qT_sb[
                        po : po + 64,
                        mt,
                        q0 + nq * 512 : q0 + (nq + 1) * 512,
                    ],
                    start=True,
                    stop=True,
                )
            ex = expool.tile([P, QG], BF16, tag="ex", name="ex")
            nc.scalar.activation(
                ex[:], sc[:], mybir.ActivationFunctionType.Exp, scale=0.125
            )
            return ex

        uls = {}
        l4s = {}

        def finish_head(qg, h, av):
            # single copy releases the av PSUM banks; row 64 is the
            # softmax denominator l (from the ones column of v_sb)
            ul = npool.tile([65, QG], F32R, tag="ul", bufs=4, name="ul")
            nc.vector.tensor_copy(ul[:], av[0:65, :])
            uls[(qg, h)] = ul
            if qg < NQG - 1:
                # batched norm: head h's l goes to partition 32h so the rb
                # broadcast matmul gets a legal base partition (0/32/64)
                if h == 0:
                    l4s[qg] = npool.tile([97, QG], F32, tag="l4", name="l4")
                    nc.gpsimd.memset(l4s[qg][:], 1.0)
                nc.vector.tensor_copy(
                    l4s[qg][32 * h : 32 * h + 1, :], ul[64:65, :]
                )
            else:
                # last query group: normalize each head immediately so the
                # tail after the final exp is as short as possible
                ln1 = npool.tile([1, QG], F32, tag="ln1", name="ln1")
                nc.scalar.activation(
                    ln1[:], ul[64:65, :], mybir.ActivationFunctionType.Ln
                )
                r1 = npool.tile([1, QG], F32R, tag="r1", name="r1")
                nc.scalar.activation(
                    r1[:], ln1[:], mybir.ActivationFunctionType.Exp, scale=-1.0
                )
                deferred.appendleft(lambda h=h: rb_mult(qg, h, r1, 0))
                if h == NHL - 1:
                    for sti in range(QG // P):
                        deferred.append(lambda sti=sti: phase_d(qg, sti))

        def rb_mult(qg, h, r4, base=None):
            mt, po = h // 2, (h % 2) * 64
            ul = uls.pop((qg, h))
            if base is not None:
                r_ap = r4
            elif h == 3:
                # matmul base partitions may only be 0/32/64; stage h3's
                # reciprocal row down to partition 0
                r3 = npool.tile([1, QG], F32R, tag="r3", name="r3")
                nc.vector.tensor_copy(r3[:], r4[96:97, :])
                r_ap, base = r3, 0
            else:
                r_ap, base = r4, 32 * h
            for nq in range(QG // 512):
                rb = ps_o.tile([P, 512], F32, tag="ops", name="rb")
                nc.tensor.matmul(
                    rb[0:64, :],
                    ones_all[base : base + 1, 0:64],
                    r_ap[base : base + 1, nq * 512 : (nq + 1) * 512],
                    start=True,
                    stop=True,
                )
                nc.vector.tensor_mul(
                    out=avT[qg][po : po + 64, mt, nq * 512 : (nq + 1) * 512],
                    in0=ul[0:64, nq * 512 : (nq + 1) * 512],
                    in1=rb[0:64, :],
                )

        def phase_d(qg, sti):
            st = qg * (QG // P) + sti
            ot = opool.tile([P, D], BF16, tag="ot", name="ot")
            for nd in range(2):
                pp = ps_o.tile([P, 512], F32, tag="ops", name="pp")
                for kt2 in range(2):
                    nc.tensor.matmul(
                        pp[:],
                        avT[qg][:, kt2, sti * P : (sti + 1) * P],
                        wo_sb[:, kt2, nd * 512 : (nd + 1) * 512],
                        start=(kt2 == 0),
                        stop=(kt2 == 1),
                    )
                nc.vector.tensor_copy(ot[:, nd * 512 : (nd + 1) * 512], pp[:])
            nc.sync.dma_start(out[st * P : (st + 1) * P, :], ot[:])

        def make_norm(qg):
            def fire():
                # one Ln + one Exp for all 4 heads' denominators:
                # 1/l = exp(-ln(l)); ACT cost is free-size-driven so the
                # [97,1024] batch costs the same as a single [1,1024].
                ls = npool.tile([97, QG], F32, tag="ls", name="ls")
                nc.scalar.activation(
                    ls[:], l4s[qg][:], mybir.ActivationFunctionType.Ln
                )
                r4 = npool.tile([97, QG], F32R, tag="r4", name="r4")
                nc.scalar.activation(
                    r4[:], ls[:], mybir.ActivationFunctionType.Exp, scale=-1.0
                )
                for h in reversed(range(NHL)):
                    deferred.appendleft(lambda h=h: rb_mult(qg, h, r4))
                for sti in range(QG // P):
                    deferred.append(lambda sti=sti: phase_d(qg, sti))


            return fire

        units = [
            (qg, h, kt)
            for qg in range(NQG)
            for h in range(NHL)
            for kt in range(ST)
        ]
        ex_tiles = {0: emit_sc_exp(*units[0])}
        av = None
        pending_norm = None
        for i, (qg, h, kt) in enumerate(units):
            if i + 1 < len(units):
                ex_tiles[i + 1] = emit_sc_exp(*units[i + 1])
            if pending_norm is not None and kt == 3:
                pending_norm()
                pending_norm = None
            if deferred:
                deferred.popleft()()
            if kt == 0:
                av = ps_av.tile([P, QG], F32, tag="av", name="av")
            ex = ex_tiles.pop(i)
            for nq in range(QG // 512):
                nc.tensor.matmul(
                    av[0:65, nq * 512 : (nq + 1) * 512],
                    v_sb[:, kt, h * 65 : h * 65 + 65],
                    ex[:, nq * 512 : (nq + 1) * 512],
                    start=(kt == 0),
                    stop=(kt == ST - 1),
                )
            if kt == ST - 1:
                finish_head(qg, h, av)
                if h == NHL - 1 and qg < NQG - 1:
                    pending_norm = make_norm(qg)

        # tail: drain the last qg's rb/mult + Wo partials
        while deferred:
            deferred.popleft()()


_NC_CACHE = None


def get_nc():
    global _NC_CACHE
    if _NC_CACHE is None:
        _NC_CACHE = build_nc()
    return _NC_CACHE


def make_in_maps(x, Wq, bq, Wk, bk, Wv, bv, Wo, bo):
    import ml_dtypes

    bf16 = ml_dtypes.bfloat16

    def w_arr(W, sl):
        # [D, DG] -> [p, kt*DG]: W[kt*128+p, n] at [p, kt, n]
        return np.ascontiguousarray(
            W[:, sl].reshape(KT_D, P, DG).transpose(1, 0, 2).reshape(P, -1)
        ).astype(bf16)

    in_maps = []
    for c in range(8):
        b, g = c // 4, c % 4
        sl = slice(g * DG, (g + 1) * DG)
        # x[b].T is [D, S]; SBUF wants [p, cb, kt, 512] with row kt*128+p,
        # col cb*512+s
        xt = (
            x[b]
            .T.reshape(KT_D, P, 4, 512)
            .transpose(1, 2, 0, 3)
            .reshape(P, -1)
            .astype(bf16)
        )
        wo_a = np.ascontiguousarray(
            Wo[sl, :].reshape(2, P, D).transpose(1, 0, 2).reshape(P, -1)
        )
        in_maps.append(
            {
                "xT": np.ascontiguousarray(xt),
                "wq": w_arr(Wq, sl),
                "wk": w_arr(Wk, sl),
                "wv": w_arr(Wv, sl),
                "wo": wo_a,
                "bq": np.ascontiguousarray(bq[sl].reshape(2, P).T),
                "bk": np.ascontiguousarray(bk[sl].reshape(2, P).T),
                "bv": np.ascontiguousarray(bv[sl].reshape(1, DG)),
            }
        )
    return in_maps


def kernel(x, Wq, bq, Wk, bk, Wv, bv, Wo, bo, _run_kwargs=None):
    from concourse.bass_utils import run_bass_kernel_spmd

    x = np.asarray(x, dtype=np.float32)
    nc = get_nc()
    in_maps = make_in_maps(
        x,
        np.asarray(Wq, np.float32),
        np.asarray(bq, np.float32),
        np.asarray(Wk, np.float32),
        np.asarray(bk, np.float32),
        np.asarray(Wv, np.float32),
        np.asarray(bv, np.float32),
        np.asarray(Wo, np.float32),
        np.asarray(bo, np.float32),
    )
    res = run_bass_kernel_spmd(
        nc, in_maps, core_ids=list(range(8)), **(_run_kwargs or {})
    )
    bo = np.asarray(bo, np.float32)
    outp = np.empty((2, S, D), dtype=np.float32)
    for b in range(2):
        acc = res.results[4 * b]["out"].astype(np.float32)
        for g in range(1, 4):
            acc = acc + res.results[4 * b + g]["out"].astype(np.float32)
        outp[b] = acc + bo[None, :]
    kernel.last_result = res
    return outp


# revision 14
# speedup vs baseline: 1.1293x; 1.1160x over previous
"""Entropy-regularized attention (standard MHA fwd) on 8 trn2 cores.

Sharding: core c -> batch b=c//4, head-group g=c%4 (4 of 16 heads).
Each core computes q/k/v for its 256-wide head-group slice, transposed-
layout attention (scores^T = K^T-stationary matmuls, exp on ACT, AV with
v-stationary producing avT), then a row-split Wo partial product.
Host sums the 4 partials per batch and adds bo (the "all-reduce").

v2 restructure vs baseline:
- x / Wq / Wk / Wv shipped as bf16 (halves input DMA), xT DMA'd in
  4 column blocks so the first projection starts ~15us earlier.
- Attention loop is software-pipelined: scores+exp for tile i+1 are
  emitted before AV of tile i, so the PE never sits on the exp latency
  and the exp pipeline (ACT) stays fed.
- All fill work (v projections, mt1 q/k projections, Wo partials of the
  previous query-group, softmax-normalize) lives in a deferred queue
  popped one small closure per loop iteration.
- Softmax denominators of all 4 heads are normalized with ONE Ln + ONE
  Exp on a [4, 1024] tile (ACT cost is free-size-driven, so per-head
  [1,1024] activations waste 4x ACT time).
- SBUF->SBUF staging copies run on the Pool engine (nc.gpsimd); PSUM
  evictions must stay on DVE (GPSIMD cannot access PSUM).
"""

import sys

for _p in ("/opt/trn_rl_repo", "/root/.axon_site/_ro/trn_rl_repo"):
    if _p not in sys.path:
        sys.path.insert(0, _p)

import numpy as np

import concourse.bass as bass
import concourse.mybir as mybir
import concourse.tile as tile
from concourse import bacc

P = 128
S = 2048  # sequence length
D = 1024  # hidden
DG = 256  # per-core head-group width (4 heads x 64)
HD = 64
NHL = 4  # heads per core
KT_D = D // P  # 8 contraction tiles for projections
ST = S // P  # 16 sequence tiles
QG = 1024  # qi group size (PSUM budget: scores 2x2 banks + av 2 + ops 2)
NQG = S // QG

F32 = mybir.dt.float32
F32R = mybir.dt.float32r
BF16 = mybir.dt.bfloat16


def build_nc():
    nc = bacc.Bacc(None, target_bir_lowering=False)

    # all inputs pre-arranged on the host into the exact SBUF layouts so
    # every DMA is a contiguous-per-partition blob (large descriptors)
    xT = nc.dram_tensor("xT", [P, 4 * KT_D * 512], BF16, kind="ExternalInput")
    wq = nc.dram_tensor("wq", [P, KT_D * DG], BF16, kind="ExternalInput")
    wk = nc.dram_tensor("wk", [P, KT_D * DG], BF16, kind="ExternalInput")
    wv = nc.dram_tensor("wv", [P, KT_D * DG], BF16, kind="ExternalInput")
    wo = nc.dram_tensor("wo", [P, 2 * D], BF16, kind="ExternalInput")
    bq = nc.dram_tensor("bq", [P, 2], F32, kind="ExternalInput")
    bk = nc.dram_tensor("bk", [P, 2], F32, kind="ExternalInput")
    bv = nc.dram_tensor("bv", [1, DG], F32R, kind="ExternalInput")
    out = nc.dram_tensor("out", [S, D], BF16, kind="ExternalOutput")

    with tile.TileContext(nc) as tc:
        _body(tc, nc, xT, wq, wk, wv, wo, bq, bk, bv, out)

    # Pin Exp/Ln to the one table set holding both: strip them from the
    # competing sets (dict order and size preserved, so act_func_set_id
    # indices stay valid). Without this the table-load pass alternates
    # exp_and_others <-> natural_log per head (~17 x 1.3us + PE stalls).
    import concourse.bacc as _bacc_mod

    _orig_tables = _bacc_mod.get_activation_tables

    def _pinned_tables(arch):
        t = _orig_tables(arch)
        for name, fns in t.items():
            if name != "natural_log_exp_and_others":
                fns.discard(mybir.ActivationFunctionType.Exp)
                fns.discard(mybir.ActivationFunctionType.Ln)
        return t

    _bacc_mod.get_activation_tables = _pinned_tables
    try:
        nc.compile()
    finally:
        _bacc_mod.get_activation_tables = _orig_tables
    return nc


def _body(tc, nc, xT, wq, wk, wv, wo, bq, bk, bv, out):
    from collections import deque
    from contextlib import ExitStack

    with ExitStack() as ctx:
        ctx.enter_context(
            nc.allow_low_precision(
                reason="bf16 matmul inputs; accumulation is fp32 PSUM"
            )
        )
        persist = ctx.enter_context(tc.tile_pool(name="persist", bufs=1))
        expool = ctx.enter_context(tc.tile_pool(name="expool", bufs=4))
        npool = ctx.enter_context(tc.tile_pool(name="npool", bufs=2))
        opool = ctx.enter_context(tc.tile_pool(name="opool", bufs=3))
        # PSUM budget (8 banks): sc 2x[128,1024]=4, av 1x[128,1024]=2,
        # ops 2x[128,512]=2. qkv/v/rb/proj tiles all use the ops slots.
        ps_sc = ctx.enter_context(tc.tile_pool(name="ps_sc", bufs=2, space="PSUM"))
        ps_av = ctx.enter_context(tc.tile_pool(name="ps_av", bufs=1, space="PSUM"))
        ps_o = ctx.enter_context(tc.tile_pool(name="ps_o", bufs=2, space="PSUM"))

        qT_sb = persist.tile([P, 2, S], BF16)
        kT_sb = persist.tile([P, 2, S], BF16)
        v_sb = persist.tile([P, ST, NHL * 65], BF16)  # 65-striped: col 64 = ones
        avT = [
            persist.tile([P, 2, QG], BF16, tag=f"avT{g}", name=f"avT{g}")
            for g in range(NQG)
        ]
        wo_sb = persist.tile([P, 2, D], BF16)
        ones_row = persist.tile([1, P], F32R)
        xT_sb = persist.tile([P, 4, KT_D, 512], BF16)
        wq_sb = persist.tile([P, KT_D, DG], BF16, tag="wq")
        wk_sb = persist.tile([P, KT_D, DG], BF16, tag="wk")
        wv_sb = persist.tile([P, KT_D, DG], BF16, tag="wv")
        bq_sb = persist.tile([P, 2], F32, tag="bq")
        bk_sb = persist.tile([P, 2], F32, tag="bk")
        bv_sb = persist.tile([1, DG], F32R, tag="bv")

        # DMAs ordered to match first-use: wk then the first xT column
        # block gate the first projection chain. All sources are
        # host-prearranged so each transfer is contiguous per partition.
        xTr = xT.rearrange("p (cb kt s) -> p cb kt s", kt=KT_D, s=512)
        nc.sync.dma_start(wk_sb[:], wk.rearrange("p (kt n) -> p kt n", n=DG))
        nc.sync.dma_start(xT_sb[:, 0], xTr[:, 0])
        nc.sync.dma_start(wq_sb[:], wq.rearrange("p (kt n) -> p kt n", n=DG))
        nc.sync.dma_start(wv_sb[:], wv.rearrange("p (kt n) -> p kt n", n=DG))
        nc.sync.dma_start(bq_sb[:], bq[:])
        nc.sync.dma_start(bk_sb[:], bk[:])
        nc.sync.dma_start(bv_sb[:], bv[:])
        nc.sync.dma_start(xT_sb[:, 1], xTr[:, 1])
        nc.sync.dma_start(xT_sb[:, 2], xTr[:, 2])
        nc.sync.dma_start(xT_sb[:, 3], xTr[:, 3])
        nc.sync.dma_start(wo_sb[:], wo.rearrange("p (kt n) -> p kt n", n=D))

        # memset can't emit float32r; stage fp32 ones and copy-cast (rounds)
        ones_f32 = persist.tile([P, P], F32)
        nc.vector.memset(ones_f32[:], 1.0)
        nc.vector.tensor_copy(ones_row[:], ones_f32[0:1, :])
        ones_all = persist.tile([P, P], F32R)
        nc.vector.tensor_copy(ones_all[:], ones_f32[:])
        nc.vector.tensor_copy(
            v_sb.rearrange("p st (h w) -> p st h w", w=65)[:, :, :, 64],
            ones_f32[:, 0:64].rearrange("p (st h) -> p st h", h=4),
        )

        # ---- projection building blocks ----
        def qk_half(ps, wsb, mt, nq, kts):
            for kt in kts:
                nc.tensor.matmul(
                    ps[:, 0:512],
                    wsb[:, kt, mt * P : (mt + 1) * P],
                    xT_sb[:, nq, kt, :],
                    start=(kt == 0),
                    stop=(kt == KT_D - 1),
                )

        def qk_evict(ps, bsb, dest, mt, nq):
            nc.vector.tensor_scalar_add(
                dest[:, mt, nq * 512 : (nq + 1) * 512],
                ps[:, 0:512],
                bsb[:, mt : mt + 1],
            )

        def qk_full(wsb, bsb, dest, mt, nq):
            ps = ps_o.tile([P, 512], F32, tag="ops", name="qkps")
            qk_half(ps, wsb, mt, nq, range(0, KT_D))
            qk_evict(ps, bsb, dest, mt, nq)

        def qk_deferred(wsb, bsb, dest, mt, nq):
            # two closures: kt 0..3, then kt 4..7 + bias eviction
            box = []

            def first():
                ps = ps_o.tile([P, 512], F32, tag="ops", name="qkps")
                qk_half(ps, wsb, mt, nq, range(0, KT_D // 2))
                box.append(ps)

            def second():
                ps = box.pop()
                qk_half(ps, wsb, mt, nq, range(KT_D // 2, KT_D))
                qk_evict(ps, bsb, dest, mt, nq)

            return [first, second]

        def v_group(st):
            ps = ps_o.tile([P, 512], F32, tag="ops", name="vps")
            for kt in range(KT_D):
                nc.tensor.matmul(
                    ps[:, 0:DG],
                    xT_sb[:, st // 4, kt, (st % 4) * P : (st % 4 + 1) * P],
                    wv_sb[:, kt, :],
                    start=(kt == 0),
                    stop=False,
                )
            nc.tensor.matmul(
                ps[:, 0:DG],
                ones_row[0:1, 0:P],
                bv_sb[0:1, :],
                start=False,
                stop=True,
            )
            nc.vector.tensor_copy(
                v_sb.rearrange("p st (h w) -> p st h w", w=65)[:, st, :, 0:64],
                ps[:, 0:DG].rearrange("p (h w) -> p h w", w=64),
            )

        # Upfront (overlaps input DMA): everything head 0/1 of qg0 needs
        # except v st>=2, which streams in through the deferred queue.
        qk_full(wk_sb, bk_sb, kT_sb, 0, 0)
        qk_full(wq_sb, bq_sb, qT_sb, 0, 0)
        v_group(0)
        v_group(1)
        qk_full(wk_sb, bk_sb, kT_sb, 0, 1)
        qk_full(wq_sb, bq_sb, qT_sb, 0, 1)
        qk_full(wk_sb, bk_sb, kT_sb, 0, 2)
        qk_full(wk_sb, bk_sb, kT_sb, 0, 3)

        deferred = deque()
        for st in range(2, ST):  # units 0..13 (v st ready 2 units early)
            deferred.append(lambda st=st: v_group(st))
        for nq in range(4):  # units 14..21
            deferred.extend(qk_deferred(wk_sb, bk_sb, kT_sb, 1, nq))
        for nq in range(2):  # units 22..25
            deferred.extend(qk_deferred(wq_sb, bq_sb, qT_sb, 1, nq))
        for nq in range(2, 4):  # units 26..29
            deferred.extend(qk_deferred(wq_sb, bq_sb, qT_sb, 0, nq))
        for nq in range(2, 4):  # units 30..33
            deferred.extend(qk_deferred(wq_sb, bq_sb, qT_sb, 1, nq))

        # ---- attention phase (software-pipelined) ----
        def emit_sc_exp(qg, h, kt):
            mt, po = h // 2, (h % 2) * 64
            q0 = qg * QG
            sc = ps_sc.tile([P, QG], F32, tag="sc", name="sc")
            for nq in range(QG // 512):
                nc.tensor.matmul(
                    sc[:, nq * 512 : (nq + 1) * 512],
                    kT_sb[po : po + 64, mt, kt * P : (kt + 1) * P],
                    qT_sb[
                        po : po + 64,
                        mt,
                        q0 + nq * 512 : q0 + (nq + 1) * 512,
                    ],
                    start=True,
                    stop=True,
                )
            ex = expool.tile([P, QG], BF16, tag="ex", name="ex")
            nc.scalar.activation(
                ex[:], sc[:], mybir.ActivationFunctionType.Exp, scale=0.125
            )
            return ex

        uls = {}
        l4s = {}

        def finish_head(qg, h, av):
            # single copy releases the av PSUM banks; row 64 is the
            # softmax denominator l (from the ones column of v_sb)
            ul = npool.tile([65, QG], F32R, tag="ul", bufs=4, name="ul")
            nc.vector.tensor_copy(ul[:], av[0:65, :])
            uls[(qg, h)] = ul
            # batched norm: head h's l goes to partition 32h so the rb
            # broadcast matmul gets a legal base partition (0/32/64)
            if h == 0:
                l4s[qg] = npool.tile([97, QG], F32, tag="l4", name="l4")
                nc.gpsimd.memset(l4s[qg][:], 1.0)
            nc.vector.tensor_copy(
                l4s[qg][32 * h : 32 * h + 1, :], ul[64:65, :]
            )

        def rb_mult_nq(qg, h, r4, r3, nq):
            mt, po = h // 2, (h % 2) * 64
            ul = uls[(qg, h)] if nq == 0 else uls.pop((qg, h))
            # matmul base partitions may only be 0/32/64; h3's reciprocal
            # row was staged down to partition 0 in r3
            r_ap, base = (r3, 0) if h == 3 else (r4, 32 * h)
            rb = ps_o.tile([P, 512], F32, tag="ops", name="rb")
            nc.tensor.matmul(
                rb[0:64, :],
                ones_all[base : base + 1, 0:64],
                r_ap[base : base + 1, nq * 512 : (nq + 1) * 512],
                start=True,
                stop=True,
            )
            nc.vector.tensor_mul(
                out=avT[qg][po : po + 64, mt, nq * 512 : (nq + 1) * 512],
                in0=ul[0:64, nq * 512 : (nq + 1) * 512],
                in1=rb[0:64, :],
            )

        def phase_d(qg, sti):
            st = qg * (QG // P) + sti
            ot = opool.tile([P, D], BF16, tag="ot", name="ot")
            for nd in range(2):
                pp = ps_o.tile([P, 512], F32, tag="ops", name="pp")
                for kt2 in range(2):
                    nc.tensor.matmul(
                        pp[:],
                        avT[qg][:, kt2, sti * P : (sti + 1) * P],
                        wo_sb[:, kt2, nd * 512 : (nd + 1) * 512],
                        start=(kt2 == 0),
                        stop=(kt2 == 1),
                    )
                nc.vector.tensor_copy(ot[:, nd * 512 : (nd + 1) * 512], pp[:])
            nc.sync.dma_start(out[st * P : (st + 1) * P, :], ot[:])

        def make_norm(qg):
            def fire():
                # one Ln + one Exp for all 4 heads' denominators:
                # 1/l = exp(-ln(l)); ACT cost is free-size-driven so the
                # [97,1024] batch costs the same as a single [1,1024].
                ls = npool.tile([97, QG], F32, tag="ls", name="ls")
                nc.scalar.activation(
                    ls[:], l4s[qg][:], mybir.ActivationFunctionType.Ln
                )
                r4 = npool.tile([97, QG], F32R, tag="r4", name="r4")
                nc.scalar.activation(
                    r4[:], ls[:], mybir.ActivationFunctionType.Exp, scale=-1.0
                )
                r3 = npool.tile([1, QG], F32R, tag="r3", name="r3")
                nc.vector.tensor_copy(r3[:], r4[96:97, :])
                # interleave by nq so the Wo partials for the first 512
                # queries start right after the 4 nq0 multiplies
                for nq in range(QG // 512):
                    for h in range(NHL):
                        deferred.append(
                            lambda h=h, nq=nq: rb_mult_nq(qg, h, r4, r3, nq)
                        )
                    for sti in range(nq * 4, nq * 4 + 4):
                        deferred.append(lambda sti=sti: phase_d(qg, sti))

            return fire

        units = [
            (qg, h, kt)
            for qg in range(NQG)
            for h in range(NHL)
            for kt in range(ST)
        ]
        ex_tiles = {0: emit_sc_exp(*units[0])}
        av = None
        pending_norm = None
        for i, (qg, h, kt) in enumerate(units):
            if i + 1 < len(units):
                ex_tiles[i + 1] = emit_sc_exp(*units[i + 1])
            if pending_norm is not None and kt == 3:
                pending_norm()
                pending_norm = None
            if deferred:
                deferred.popleft()()
            if kt == 0:
                av = ps_av.tile([P, QG], F32, tag="av", name="av")
            ex = ex_tiles.pop(i)
            for nq in range(QG // 512):
                nc.tensor.matmul(
                    av[0:65, nq * 512 : (nq + 1) * 512],
                    v_sb[:, kt, h * 65 : h * 65 + 65],
                    ex[:, nq * 512 : (nq + 1) * 512],
                    start=(kt == 0),
                    stop=(kt == ST - 1),
                )
            if kt == ST - 1:
                finish_head(qg, h, av)
                if h == NHL - 1:
                    pending_norm = make_norm(qg)

        # tail: fire the last qg's norm, then drain its rb/mult + Wo
        pending_norm()
        while deferred:
            deferred.popleft()()


_NC_CACHE = None


def get_nc():
    global _NC_CACHE
    if _NC_CACHE is None:
        _NC_CACHE = build_nc()
    return _NC_CACHE


def make_in_maps(x, Wq, bq, Wk, bk, Wv, bv, Wo, bo):
    import ml_dtypes

    bf16 = ml_dtypes.bfloat16

    def w_arr(W, sl):
        # [D, DG] -> [p, kt*DG]: W[kt*128+p, n] at [p, kt, n]
        return np.ascontiguousarray(
            W[:, sl].reshape(KT_D, P, DG).transpose(1, 0, 2).reshape(P, -1)
        ).astype(bf16)

    in_maps = []
    for c in range(8):
        b, g = c // 4, c % 4
        sl = slice(g * DG, (g + 1) * DG)
        # x[b].T is [D, S]; SBUF wants [p, cb, kt, 512] with row kt*128+p,
        # col cb*512+s
        xt = (
            x[b]
            .T.reshape(KT_D, P, 4, 512)
            .transpose(1, 2, 0, 3)
            .reshape(P, -1)
            .astype(bf16)
        )
        wo_a = np.ascontiguousarray(
            Wo[sl, :].reshape(2, P, D).transpose(1, 0, 2).reshape(P, -1)
        ).astype(bf16)
        in_maps.append(
            {
                "xT": np.ascontiguousarray(xt),
                "wq": w_arr(Wq, sl),
                "wk": w_arr(Wk, sl),
                "wv": w_arr(Wv, sl),
                "wo": wo_a,
                "bq": np.ascontiguousarray(bq[sl].reshape(2, P).T),
                "bk": np.ascontiguousarray(bk[sl].reshape(2, P).T),
                "bv": np.ascontiguousarray(bv[sl].reshape(1, DG)),
            }
        )
    return in_maps


def kernel(x, Wq, bq, Wk, bk, Wv, bv, Wo, bo, _run_kwargs=None):
    from concourse.bass_utils import run_bass_kernel_spmd

    x = np.asarray(x, dtype=np.float32)
    nc = get_nc()
    in_maps = make_in_maps(
        x,
        np.asarray(Wq, np.float32),
        np.asarray(bq, np.float32),
        np.asarray(Wk, np.float32),
        np.asarray(bk, np.float32),
        np.asarray(Wv, np.float32),
        np.asarray(bv, np.float32),
        np.asarray(Wo, np.float32),
        np.asarray(bo, np.float32),
    )
    res = run_bass_kernel_spmd(
        nc, in_maps, core_ids=list(range(8)), **(_run_kwargs or {})
    )
    bo = np.asarray(bo, np.float32)
    outp = np.empty((2, S, D), dtype=np.float32)
    for b in range(2):
        acc = res.results[4 * b]["out"].astype(np.float32)
        for g in range(1, 4):
            acc = acc + res.results[4 * b + g]["out"].astype(np.float32)
        outp[b] = acc + bo[None, :]
    kernel.last_result = res
    return outp
